# revision 9
# baseline (speedup 1.0000x reference)
"""Bilateral filter (7x7, dilation 1) Trainium2 Bass kernel.

Problem: input [2, 18, 1024, 1024] f32.
  filterable = input[:, :8]; params = -(input[:, 8:]**2)
  range coeffs = params[:, :8], sx = params[:, 8], sy = params[:, 9]
  out[c] = sum_taps w * f_c(shifted) / sum_taps w, c < 3
  w = exp(sum_c r_c (fn_c - f_c)^2 + sx dx^2 + sy dy^2), OOB taps masked.

Sharding: data-parallel over (batch, H): 8 cores, each gets 256 rows of one
batch image (+3 halo rows each side, sentinel-padded host-side).  Out-of-image
taps get weight exactly 0 because the sentinel (1e18) drives the quadratic
form to -huge and exp underflows to +0.

Per-core layout: H rows on partitions (128 x 2 blocks), W in chunks of 256 on
the free axis with the 8 filterable channels interleaved (x*8+c).  Row shifts
(oy) come from 7 row-shifted tile copies; column shifts (ox) are free-axis
offsets into the 6-column halo.

Engine split per tap: DVE sub/reduce/adds, ACT square/exp, GPSIMD r*d^2.
"""

import sys

if "/opt/trn_rl_repo" not in sys.path:
    sys.path.insert(0, "/opt/trn_rl_repo")

import numpy as np

import concourse.bass as bass
import concourse.mybir as mybir
from concourse.bacc import Bacc
from concourse.tile import TileContext

FP32 = mybir.dt.float32

B, C_ALL, H, W = 2, 18, 1024, 1024
CF = 8                      # filterable channels
CO = 3                      # output channels
KS, RAD = 7, 3
HC = H * B // 8             # 256 output rows per core
HIN = HC + 2 * RAD          # 262 input rows per core (halo padded host-side)
WC = 256                    # W chunk
NW = W // WC                # 4
NHB = HC // 128             # 2
SENT = 1.0e18               # sentinel padding value -> tap weight exp(-huge)=0
D2 = [9.0, 4.0, 1.0, 0.0, 1.0, 4.0, 9.0]   # (k-3)^2 for k in 0..6
D2IDX = [3, 2, 1, 0, 1, 2, 3]              # index into [0,1,4,9]
D2VALS = [0.0, 1.0, 4.0, 9.0]

_CACHED = {}
TAP_SET = None   # optional [(i,j)] subset for debugging


def _ilv(ap, n, c=CF):
    """View flat [128, n*c] region as [128, n, c] (channel-interleaved)."""
    return ap.rearrange("p (x c) -> p x c", c=c)


def build_nc(macros=None):
    nc = Bacc()
    x = nc.dram_tensor("x", [C_ALL, HIN, W], FP32, kind="ExternalInput")
    y = nc.dram_tensor("y", [CO, HC, W], FP32, kind="ExternalOutput")

    if macros is None:
        macros = [(hb, wck) for hb in range(NHB) for wck in range(NW)]
    with TileContext(nc) as tc:
        with (
            tc.tile_pool(name="fpool", bufs=1) as fpool,
            tc.tile_pool(name="cpool", bufs=1) as cpool,
            tc.tile_pool(name="dpool", bufs=3) as dpool,
            tc.tile_pool(name="spool", bufs=3) as spool,
            tc.tile_pool(name="ppool", bufs=1, space="PSUM") as ppool,
        ):
            for hb, wcki in macros:
                _macro(nc, tc, x, y, fpool, cpool, dpool, spool, ppool, hb, wcki)
    nc.compile()
    return nc


def _macro(nc, tc, x, y, fpool, cpool, dpool, spool, ppool, hb, wck):
    w0 = wck * WC
    r0 = hb * 128
    wtile = WC + 2 * RAD
    # tile col t  <->  image col w0 - 3 + t
    lo = RAD if wck == 0 else 0
    hi = wtile - RAD if wck == NW - 1 else wtile

    # ---- load + interleave the 7 row-shifted filterable tile sets ----
    F = []
    for oy in range(KS):
        Fi = fpool.tile([128, wtile * CF], FP32, tag=f"F{oy}", bufs=2,
                        name=f"F{oy}_{hb}_{wck}")
        for c in range(CF):
            pl = fpool.tile([128, wtile], FP32, tag="pl", bufs=3,
                            name=f"pl_{hb}_{wck}_{oy}_{c}")
            if lo > 0:
                nc.gpsimd.memset(pl[:, 0:lo], SENT)
            if hi < wtile:
                nc.gpsimd.memset(pl[:, hi:wtile], SENT)
            nc.sync.dma_start(
                out=pl[:, lo:hi],
                in_=x[c, r0 + oy : r0 + oy + 128, w0 - RAD + lo : w0 - RAD + hi],
            )
            # interleave: Fi[p, t*8+c] = pl[p, t]   (ACT, strided out)
            nc.scalar.copy(_ilv(Fi[:], wtile)[:, :, c], pl[:])
        F.append(Fi)
    Fc = _ilv(F[RAD][:, RAD * CF : (RAD + WC) * CF], WC)      # center view

    # ---- params: R (interleaved), sx2, sy2 ----
    R = cpool.tile([128, WC * CF], FP32, tag="R", name=f"R_{hb}_{wck}")
    for c in range(CF):
        pp = fpool.tile([128, WC], FP32, tag="pp", bufs=2,
                        name=f"pp_{hb}_{wck}_{c}")
        nc.sync.dma_start(
            out=pp[:], in_=x[CF + c, r0 + RAD : r0 + RAD + 128, w0 : w0 + WC])
        nc.vector.scalar_tensor_tensor(
            _ilv(R[:], WC)[:, :, c], pp[:], -1.0, pp[:],
            mybir.AluOpType.mult, mybir.AluOpType.mult)
    sxy2 = cpool.tile([128, 2 * WC], FP32, tag="sxy2", name=f"sxy2_{hb}_{wck}")
    for k in range(2):
        pp = fpool.tile([128, WC], FP32, tag="pp", bufs=2,
                        name=f"pps_{hb}_{wck}_{k}")
        nc.sync.dma_start(
            out=pp[:], in_=x[2 * CF + k, r0 + RAD : r0 + RAD + 128, w0 : w0 + WC])
        nc.vector.scalar_tensor_tensor(
            sxy2[:, k * WC : (k + 1) * WC], pp[:], -1.0, pp[:],
            mybir.AluOpType.mult, mybir.AluOpType.mult)
    sx2 = sxy2[:, 0:WC]
    sy2 = sxy2[:, WC : 2 * WC]

    # ---- Asp[a][b] = a*sx2 + b*sy2  (spatial log-weight, 16 combos) ----
    Ab = spool.tile([128, 4 * WC], FP32, tag="Ab", bufs=2, name=f"Ab_{hb}_{wck}")
    for bi, bval in enumerate(D2VALS):
        nc.vector.tensor_scalar_mul(
            Ab[:, bi * WC : (bi + 1) * WC], sy2, float(bval))
    Asp = ppool.tile([128, 16 * WC], FP32, tag="Asp", name=f"Asp_{hb}_{wck}")
    for ai, aval in enumerate(D2VALS):
        for bi in range(4):
            nc.vector.scalar_tensor_tensor(
                Asp[:, (ai * 4 + bi) * WC : (ai * 4 + bi + 1) * WC],
                sx2, float(aval), Ab[:, bi * WC : (bi + 1) * WC],
                mybir.AluOpType.mult, mybir.AluOpType.add)

    # ---- accumulators ----
    acc = cpool.tile([128, WC * CO], FP32, tag="acc", name=f"acc_{hb}_{wck}")
    wsum = cpool.tile([128, WC], FP32, tag="wsum", name=f"wsum_{hb}_{wck}")
    nc.gpsimd.memset(acc[:], 0.0)
    nc.gpsimd.memset(wsum[:], 0.0)

    # ---- 49 taps ----
    taps = TAP_SET if TAP_SET is not None else [(i, j) for i in range(KS) for j in range(KS)]
    for i, j in taps:            # oy = i - 3, ox = j - 3
        if True:
            Fi = F[i]
            sh = _ilv(Fi[:, j * CF : (j + WC) * CF], WC)     # shifted read
            d = dpool.tile([128, WC * CF], FP32, tag="d",
                           name=f"d_{hb}_{wck}_{i}_{j}")
            nc.vector.tensor_sub(_ilv(d[:], WC), sh, Fc)
            nc.scalar.activation(d[:], d[:], mybir.ActivationFunctionType.Square)
            nc.gpsimd.tensor_mul(d[:], R[:], d[:])
            s = spool.tile([128, WC], FP32, tag="s",
                           name=f"s_{hb}_{wck}_{i}_{j}")
            nc.vector.tensor_reduce(s[:], _ilv(d[:], WC),
                                    axis=mybir.AxisListType.X,
                                    op=mybir.AluOpType.add)
            k = (D2IDX[j] * 4 + D2IDX[i]) * WC
            nc.vector.tensor_add(s[:], s[:], Asp[:, k : k + WC])
            w_t = spool.tile([128, WC], FP32, tag="w",
                             name=f"w_{hb}_{wck}_{i}_{j}")
            nc.scalar.activation(w_t[:], s[:], mybir.ActivationFunctionType.Exp)
            nc.vector.tensor_add(wsum[:], wsum[:], w_t[:])
            t3 = spool.tile([128, WC * CO], FP32, tag="t3",
                            name=f"t3_{hb}_{wck}_{i}_{j}")
            w_b = w_t[:].unsqueeze(2).broadcast_to([128, WC, CO])
            f3 = _ilv(Fi[:, j * CF : (j + WC) * CF], WC)[:, :, 0:CO]
            nc.vector.tensor_mul(_ilv(t3[:], WC, CO), w_b, f3)
            nc.vector.tensor_add(acc[:], acc[:], t3[:])

    # ---- out = acc / wsum ----
    rec = spool.tile([128, WC], FP32, tag="s", name=f"rec_{hb}_{wck}")
    nc.vector.reciprocal(rec[:], wsum[:])
    out3 = spool.tile([128, WC * CO], FP32, tag="t3", name=f"out3_{hb}_{wck}")
    rec_b = rec[:].unsqueeze(2).broadcast_to([128, WC, CO])
    nc.vector.tensor_mul(_ilv(out3[:], WC, CO), rec_b, _ilv(acc[:], WC, CO))
    for c in range(CO):
        oc = spool.tile([128, WC], FP32, tag="oc", name=f"oc_{hb}_{wck}_{c}")
        nc.scalar.copy(oc[:], _ilv(out3[:], WC, CO)[:, :, c])
        nc.sync.dma_start(out=y[c, r0 : r0 + 128, w0 : w0 + WC], in_=oc[:])


def shard_inputs(input):
    """input [2,18,1024,1024] -> 8 per-core slabs [18, 262, 1024]."""
    input = np.asarray(input, dtype=np.float32)
    per_b = 4
    rows = H // per_b
    in_maps = []
    for core in range(8):
        b, q = divmod(core, per_b)
        r0 = q * rows
        slab = np.full((C_ALL, HIN, W), SENT, dtype=np.float32)
        s_lo = max(r0 - RAD, 0)
        s_hi = min(r0 + rows + RAD, H)
        slab[:, s_lo - (r0 - RAD) : s_hi - (r0 - RAD), :] = input[b, :, s_lo:s_hi, :]
        in_maps.append({"x": np.ascontiguousarray(slab)})
    return in_maps


def assemble(results):
    out = np.empty((B, CO, H, W), dtype=np.float32)
    rows = H // 4
    for core in range(8):
        b, q = divmod(core, 4)
        out[b, :, q * rows : (q + 1) * rows, :] = results[core]["y"]
    return out


def kernel(input):
    from concourse.bass_utils import run_bass_kernel_spmd

    if "nc" not in _CACHED:
        _CACHED["nc"] = build_nc()
    in_maps = shard_inputs(input)
    res = run_bass_kernel_spmd(_CACHED["nc"], in_maps, list(range(8)))
    return assemble(res.results)


# revision 10
# speedup vs baseline: 1.0087x; 1.0087x over previous
"""Bilateral filter (7x7, dilation 1) Trainium2 Bass kernel.

Problem: input [2, 18, 1024, 1024] f32.
  filterable = input[:, :8]; params = -(input[:, 8:]**2)
  range coeffs = params[:, :8], sx = params[:, 8], sy = params[:, 9]
  out[c] = sum_taps w * f_c(shifted) / sum_taps w, c < 3
  w = exp(sum_c r_c (fn_c - f_c)^2 + sx dx^2 + sy dy^2), OOB taps masked.

Sharding: data-parallel over (batch, H): 8 cores, each gets 256 rows of one
batch image (+3 halo rows each side, sentinel-padded host-side).  Out-of-image
taps get weight exactly 0 because the sentinel (1e18) drives the quadratic
form to -huge and exp underflows to +0.

Per-core layout: H rows on partitions (128 x 2 blocks), W in chunks of 256 on
the free axis with the 8 filterable channels interleaved (x*8+c).  Row shifts
(oy) come from 7 row-shifted tile copies; column shifts (ox) are free-axis
offsets into the 6-column halo.

Engine split per tap: DVE sub/reduce/adds, ACT square/exp, GPSIMD r*d^2.
"""

import sys

if "/opt/trn_rl_repo" not in sys.path:
    sys.path.insert(0, "/opt/trn_rl_repo")

import numpy as np

import concourse.bass as bass
import concourse.mybir as mybir
from concourse.bacc import Bacc
from concourse.tile import TileContext

FP32 = mybir.dt.float32

B, C_ALL, H, W = 2, 18, 1024, 1024
CF = 8                      # filterable channels
CO = 3                      # output channels
KS, RAD = 7, 3
HC = H * B // 8             # 256 output rows per core
HIN = HC + 2 * RAD          # 262 input rows per core (halo padded host-side)
WC = 256                    # W chunk
NW = W // WC                # 4
NHB = HC // 128             # 2
SENT = 1.0e18               # sentinel padding value -> tap weight exp(-huge)=0
D2 = [9.0, 4.0, 1.0, 0.0, 1.0, 4.0, 9.0]   # (k-3)^2 for k in 0..6
D2IDX = [3, 2, 1, 0, 1, 2, 3]              # index into [0,1,4,9]
D2VALS = [0.0, 1.0, 4.0, 9.0]

_CACHED = {}
TAP_SET = None   # optional [(i,j)] subset for debugging


def _ilv(ap, n, c=CF):
    """View flat [128, n*c] region as [128, n, c] (channel-interleaved)."""
    return ap.rearrange("p (x c) -> p x c", c=c)


def build_nc(macros=None):
    nc = Bacc()
    x = nc.dram_tensor("x", [C_ALL, HIN, W], FP32, kind="ExternalInput")
    y = nc.dram_tensor("y", [CO, HC, W], FP32, kind="ExternalOutput")

    if macros is None:
        macros = [(hb, wck) for hb in range(NHB) for wck in range(NW)]
    with TileContext(nc) as tc:
        with (
            tc.tile_pool(name="fpool", bufs=1) as fpool,
            tc.tile_pool(name="cpool", bufs=1) as cpool,
            tc.tile_pool(name="dpool", bufs=5) as dpool,
            tc.tile_pool(name="spool", bufs=4) as spool,
            tc.tile_pool(name="ppool", bufs=1, space="PSUM") as ppool,
        ):
            for hb, wcki in macros:
                _macro(nc, tc, x, y, fpool, cpool, dpool, spool, ppool, hb, wcki)
    nc.compile()
    return nc


def _macro(nc, tc, x, y, fpool, cpool, dpool, spool, ppool, hb, wck):
    w0 = wck * WC
    r0 = hb * 128
    wtile = WC + 2 * RAD
    # tile col t  <->  image col w0 - 3 + t
    lo = RAD if wck == 0 else 0
    hi = wtile - RAD if wck == NW - 1 else wtile

    # ---- load + interleave the 7 row-shifted filterable tile sets ----
    F = []
    for oy in range(KS):
        Fi = fpool.tile([128, wtile * CF], FP32, tag=f"F{oy}", bufs=1,
                        name=f"F{oy}_{hb}_{wck}")
        for c in range(CF):
            pl = fpool.tile([128, wtile], FP32, tag="pl", bufs=3,
                            name=f"pl_{hb}_{wck}_{oy}_{c}")
            if lo > 0:
                nc.gpsimd.memset(pl[:, 0:lo], SENT)
            if hi < wtile:
                nc.gpsimd.memset(pl[:, hi:wtile], SENT)
            nc.sync.dma_start(
                out=pl[:, lo:hi],
                in_=x[c, r0 + oy : r0 + oy + 128, w0 - RAD + lo : w0 - RAD + hi],
            )
            # interleave: Fi[p, t*8+c] = pl[p, t]   (ACT, strided out)
            nc.scalar.copy(_ilv(Fi[:], wtile)[:, :, c], pl[:])
        F.append(Fi)
    Fc = _ilv(F[RAD][:, RAD * CF : (RAD + WC) * CF], WC)      # center view

    # ---- params: R (interleaved), sx2, sy2 ----
    R = cpool.tile([128, WC * CF], FP32, tag="R", name=f"R_{hb}_{wck}")
    for c in range(CF):
        pp = fpool.tile([128, WC], FP32, tag="pp", bufs=2,
                        name=f"pp_{hb}_{wck}_{c}")
        nc.sync.dma_start(
            out=pp[:], in_=x[CF + c, r0 + RAD : r0 + RAD + 128, w0 : w0 + WC])
        nc.vector.scalar_tensor_tensor(
            _ilv(R[:], WC)[:, :, c], pp[:], -1.0, pp[:],
            mybir.AluOpType.mult, mybir.AluOpType.mult)
    sxy2 = cpool.tile([128, 2 * WC], FP32, tag="sxy2", name=f"sxy2_{hb}_{wck}")
    for k in range(2):
        pp = fpool.tile([128, WC], FP32, tag="pp", bufs=2,
                        name=f"pps_{hb}_{wck}_{k}")
        nc.sync.dma_start(
            out=pp[:], in_=x[2 * CF + k, r0 + RAD : r0 + RAD + 128, w0 : w0 + WC])
        nc.vector.scalar_tensor_tensor(
            sxy2[:, k * WC : (k + 1) * WC], pp[:], -1.0, pp[:],
            mybir.AluOpType.mult, mybir.AluOpType.mult)
    sx2 = sxy2[:, 0:WC]
    sy2 = sxy2[:, WC : 2 * WC]

    # ---- Asp[a][b] = a*sx2 + b*sy2  (spatial log-weight, 16 combos) ----
    Ab = spool.tile([128, 4 * WC], FP32, tag="Ab", bufs=2, name=f"Ab_{hb}_{wck}")
    for bi, bval in enumerate(D2VALS):
        nc.vector.tensor_scalar_mul(
            Ab[:, bi * WC : (bi + 1) * WC], sy2, float(bval))
    Asp = cpool.tile([128, 16 * WC], FP32, tag="Asp", name=f"Asp_{hb}_{wck}")
    for ai, aval in enumerate(D2VALS):
        for bi in range(4):
            nc.vector.scalar_tensor_tensor(
                Asp[:, (ai * 4 + bi) * WC : (ai * 4 + bi + 1) * WC],
                sx2, float(aval), Ab[:, bi * WC : (bi + 1) * WC],
                mybir.AluOpType.mult, mybir.AluOpType.add)

    # ---- accumulators ----
    acc = cpool.tile([128, WC * CO], FP32, tag="acc", name=f"acc_{hb}_{wck}")
    wsum = cpool.tile([128, WC], FP32, tag="wsum", name=f"wsum_{hb}_{wck}")
    nc.gpsimd.memset(acc[:], 0.0)
    nc.gpsimd.memset(wsum[:], 0.0)

    # ---- 49 taps ----
    taps = TAP_SET if TAP_SET is not None else [(i, j) for i in range(KS) for j in range(KS)]
    for i, j in taps:            # oy = i - 3, ox = j - 3
        if True:
            Fi = F[i]
            sh = _ilv(Fi[:, j * CF : (j + WC) * CF], WC)     # shifted read
            d = dpool.tile([128, WC * CF], FP32, tag="d",
                           name=f"d_{hb}_{wck}_{i}_{j}")
            nc.vector.tensor_sub(_ilv(d[:], WC), sh, Fc)
            nc.scalar.activation(d[:], d[:], mybir.ActivationFunctionType.Square)
            nc.gpsimd.tensor_mul(d[:], R[:], d[:])
            s = spool.tile([128, WC], FP32, tag="s",
                           name=f"s_{hb}_{wck}_{i}_{j}")
            nc.vector.tensor_reduce(s[:], _ilv(d[:], WC),
                                    axis=mybir.AxisListType.X,
                                    op=mybir.AluOpType.add)
            k = (D2IDX[j] * 4 + D2IDX[i]) * WC
            nc.vector.tensor_add(s[:], s[:], Asp[:, k : k + WC])
            w_t = spool.tile([128, WC], FP32, tag="w",
                             name=f"w_{hb}_{wck}_{i}_{j}")
            nc.scalar.activation(w_t[:], s[:], mybir.ActivationFunctionType.Exp)
            nc.vector.tensor_add(wsum[:], wsum[:], w_t[:])
            t3 = spool.tile([128, WC * CO], FP32, tag="t3",
                            name=f"t3_{hb}_{wck}_{i}_{j}")
            w_b = w_t[:].unsqueeze(2).broadcast_to([128, WC, CO])
            f3 = _ilv(Fi[:, j * CF : (j + WC) * CF], WC)[:, :, 0:CO]
            nc.vector.tensor_mul(_ilv(t3[:], WC, CO), w_b, f3)
            nc.vector.tensor_add(acc[:], acc[:], t3[:])

    # ---- out = acc / wsum ----
    rec = spool.tile([128, WC], FP32, tag="s", name=f"rec_{hb}_{wck}")
    nc.vector.reciprocal(rec[:], wsum[:])
    out3 = spool.tile([128, WC * CO], FP32, tag="t3", name=f"out3_{hb}_{wck}")
    rec_b = rec[:].unsqueeze(2).broadcast_to([128, WC, CO])
    nc.vector.tensor_mul(_ilv(out3[:], WC, CO), rec_b, _ilv(acc[:], WC, CO))
    for c in range(CO):
        oc = spool.tile([128, WC], FP32, tag="oc", name=f"oc_{hb}_{wck}_{c}")
        nc.scalar.copy(oc[:], _ilv(out3[:], WC, CO)[:, :, c])
        nc.sync.dma_start(out=y[c, r0 : r0 + 128, w0 : w0 + WC], in_=oc[:])


def shard_inputs(input):
    """input [2,18,1024,1024] -> 8 per-core slabs [18, 262, 1024]."""
    input = np.asarray(input, dtype=np.float32)
    per_b = 4
    rows = H // per_b
    in_maps = []
    for core in range(8):
        b, q = divmod(core, per_b)
        r0 = q * rows
        slab = np.full((C_ALL, HIN, W), SENT, dtype=np.float32)
        s_lo = max(r0 - RAD, 0)
        s_hi = min(r0 + rows + RAD, H)
        slab[:, s_lo - (r0 - RAD) : s_hi - (r0 - RAD), :] = input[b, :, s_lo:s_hi, :]
        in_maps.append({"x": np.ascontiguousarray(slab)})
    return in_maps


def assemble(results):
    out = np.empty((B, CO, H, W), dtype=np.float32)
    rows = H // 4
    for core in range(8):
        b, q = divmod(core, 4)
        out[b, :, q * rows : (q + 1) * rows, :] = results[core]["y"]
    return out


def kernel(input):
    from concourse.bass_utils import run_bass_kernel_spmd

    if "nc" not in _CACHED:
        _CACHED["nc"] = build_nc()
    in_maps = shard_inputs(input)
    res = run_bass_kernel_spmd(_CACHED["nc"], in_maps, list(range(8)))
    return assemble(res.results)
